# revision 1
# baseline (speedup 1.0000x reference)
"""Grouped-Query Attention on 8 Trainium2 NeuronCores.

Sharding: core c handles (batch b = c//4, query-head group g = c%4).
Each core computes its group's Q projection (256 cols of W_Q), the
group-shared K/V projections, 4 heads of attention over the full
sequence, and a partial output projection against the group's 256 rows
of W_O. The host sums the 4 group partials per batch (the "all-reduce")
and adds b_O.

On-core dataflow (all matmuls bf16 operands, fp32 PSUM accumulate):
  xT   = transpose(x)                  PE transpose, fp32 -> bf16 on evac
  QT   = W_Q^T x  (q-dim on partitions), + b_Q on evac
  KT   = W_K^T x  (d_k on partitions), + b_K on evac
  V    = x W_V    (natural [t, d_k])
  S^T  = KT_h^T @ QT_h   per head, [t, s] layout
  P^T  = exp(S^T / 8)    ScalarE, PSUM -> SBUF bf16
  ctx  = P^T_chunk^T @ [V | 1]   natural [s, d_k+1]; col 64 = softmax denom
  ctx /= denom; transpose -> ctxT; out = ctxT^T @ W_O (partial, fp32 out)

b_V and b_O are applied on the host: b_V adds exactly
(tile(b_V) @ W_O_g) to every output row (softmax weights sum to 1).
"""

import numpy as np

S = 2048
DM = 1024
G = 4
H = 4  # heads per group
DK = 64
GQ = 256  # query width per group
B = 2
NK = DM // 128  # 8 contraction chunks
NT = S // 128  # 16 token chunks
SBLK = 512
NSB = S // SBLK  # 4 query super-blocks

_CACHED = {}


def _split_sync_waits(nc, drain_max=1, other_max=1):
    """This walrus build has a single sync-wait slot on CTRL-class
    instructions (Drain/NoOp); Tile's exit drain collects 3+. Move the
    excess onto preceding single-wait NOPs on the same engine."""
    import concourse.mybir as mybir
    import bass_rust

    n_split = 0
    for f in nc.m.functions:
        for bb in f.blocks:
            out = []
            changed = False
            for inst in bb.instructions:
                si = getattr(inst, "sync_info", None)
                limit = drain_max if type(inst).__name__ in ("InstDrain", "InstNoOp") else other_max
                if si is not None and len(si.on_wait) > limit:
                    waits = list(si.on_wait)
                    keep = waits[-limit:] if limit else []
                    head = waits[: len(waits) - limit]
                    for w in head:
                        out.append(
                            mybir.InstNoOp(
                                name=f"{inst.name}-wsp{n_split}",
                                engine=inst.engine,
                                sync_info=mybir.SyncInfo(on_wait=[w], on_update=[]),
                                bass_nofuse=True,
                            )
                        )
                        n_split += 1
                    inst.sync_info = bass_rust.SyncInfo(on_wait=keep, on_update=si.on_update)
                    changed = True
                out.append(inst)
            if changed:
                bb.instructions = out
    return n_split


def _build_nc(iters=1, fp32_tr=True):
    import concourse.bass as bass
    import concourse.mybir as mybir
    import concourse.tile as tile
    from concourse import masks

    F32 = mybir.dt.float32
    BF = mybir.dt.bfloat16

    nc = bass.Bass("TRN2", target_bir_lowering=False, debug=False, num_devices=8)
    x = nc.dram_tensor("x", [S, DM], F32, kind="ExternalInput")
    wq = nc.dram_tensor("wq", [DM, GQ], F32, kind="ExternalInput")
    wk = nc.dram_tensor("wk", [DM, DK], F32, kind="ExternalInput")
    wv = nc.dram_tensor("wv", [DM, DK], F32, kind="ExternalInput")
    wo = nc.dram_tensor("wo", [GQ, DM], F32, kind="ExternalInput")
    bq = nc.dram_tensor("bq", [GQ], F32, kind="ExternalInput")
    bk = nc.dram_tensor("bk", [DK], F32, kind="ExternalInput")
    out = nc.dram_tensor("out", [S, DM], F32, kind="ExternalOutput")

    with tile.TileContext(nc) as tc:
        with (
            tc.tile_pool(name="const", bufs=1) as cpool,
            tc.tile_pool(name="wstg", bufs=3) as wstg,
            tc.tile_pool(name="wts", bufs=1) as wts,
            tc.tile_pool(name="xin", bufs=2) as xin,
            tc.tile_pool(name="acts", bufs=1) as acts,
            tc.tile_pool(name="outp", bufs=3) as outp,
            tc.tile_pool(name="ps_sc", bufs=2, space="PSUM") as ps_sc,
            tc.tile_pool(name="ps_proj", bufs=2, space="PSUM") as ps_proj,
            tc.tile_pool(name="ps_ctx", bufs=2, space="PSUM") as ps_ctx,
        ):
            def _pipeline():
                # ---- constants ----
                ident_f = cpool.tile([128, 128], F32)
                masks.make_identity(nc, ident_f[:])
                ident_b = cpool.tile([128, 128], BF)
                masks.make_identity(nc, ident_b[:])
                bq_t = cpool.tile([128, 2], F32)
                for m in range(2):
                    nc.sync.dma_start(bq_t[:, m : m + 1], bq[m * 128 : (m + 1) * 128])
                bk_t = cpool.tile([64, 1], F32)
                nc.sync.dma_start(bk_t[:], bk[:])

                # ---- weights: stage fp32 (batched 3D-AP DMAs), cast to bf16 ----
                wq_bf = wts.tile([128, NK * GQ], BF)  # chunk k at cols [k*GQ, (k+1)*GQ)
                kv_bf = wts.tile([128, NK * DK * 2], BF)  # wk at k*64, wv at 512+k*64
                wo_bf = wts.tile([128, 2 * DM], BF)  # chunk cj at cols [cj*DM, ...)

                stg = wstg.tile([128, NK * GQ], F32, tag="stg")
                nc.sync.dma_start(
                    stg[:].rearrange("p (k q) -> p k q", q=GQ),
                    wq[:].rearrange("(k p) q -> p k q", p=128),
                )
                nc.vector.tensor_copy(wq_bf[:], stg[:])

                stg2 = wstg.tile([128, NK * GQ], F32, tag="stg")
                nc.sync.dma_start(
                    stg2[:, :512].rearrange("p (k q) -> p k q", q=DK),
                    wk[:].rearrange("(k p) q -> p k q", p=128),
                )
                nc.sync.dma_start(
                    stg2[:, 512:1024].rearrange("p (k q) -> p k q", q=DK),
                    wv[:].rearrange("(k p) q -> p k q", p=128),
                )
                nc.vector.tensor_copy(kv_bf[:], stg2[:, : NK * DK * 2])

                stg3 = wstg.tile([128, NK * GQ], F32, tag="stg")
                nc.sync.dma_start(
                    stg3[:].rearrange("p (c n) -> p c n", n=DM),
                    wo[:].rearrange("(c p) n -> p c n", p=128),
                )
                nc.vector.tensor_copy(wo_bf[:], stg3[:])

                # ---- x load (batched) + transpose (xT[:, k*S + s], bf16) ----
                xT = acts.tile([128, NK * S], BF)
                for sg in range(NSB):
                    xf = xin.tile([128, 4 * DM], F32, tag="xf")
                    nc.sync.dma_start(
                        xf[:].rearrange("p (c d) -> p c d", d=DM),
                        x[sg * SBLK : (sg + 1) * SBLK, :].rearrange("(c p) d -> p c d", p=128),
                    )
                    if fp32_tr:
                        for k in range(NK):
                            ps = ps_proj.tile([128, SBLK], F32, tag="p")
                            for i in range(4):
                                nc.tensor.transpose(
                                    ps[:, i * 128 : (i + 1) * 128],
                                    xf[:, i * DM + k * 128 : i * DM + (k + 1) * 128],
                                    ident_f[:],
                                )
                            nc.vector.tensor_copy(
                                xT[:, k * S + sg * SBLK : k * S + (sg + 1) * SBLK], ps[:]
                            )
                    else:
                        xbf = xin.tile([128, 4 * DM], BF, tag="xbf", bufs=1)
                        nc.vector.tensor_copy(xbf[:], xf[:])
                        for k in range(NK):
                            ps = ps_proj.tile([128, 2 * SBLK], BF, tag="p", name="psx")
                            for i in range(4):
                                nc.tensor.transpose(
                                    ps[:, i * 128 : (i + 1) * 128],
                                    xbf[:, i * DM + k * 128 : i * DM + (k + 1) * 128],
                                    ident_b[:],
                                )
                            nc.vector.tensor_copy(
                                xT[:, k * S + sg * SBLK : k * S + (sg + 1) * SBLK], ps[:, :SBLK]
                            )

                # ---- projections ----
                QT = acts.tile([128, 2 * S], BF)  # m-tile m at cols [m*S, ...): heads 2m, 2m+1
                KT = acts.tile([128, S], BF)  # rows 64-127 duplicate 0-63 (row-tiled scores)
                Vb = acts.tile([128, NT * (DK + 1)], BF)  # [V | ones] per token chunk

                for sg in range(NSB):
                    ps = ps_proj.tile([128, SBLK], F32, tag="p")
                    for k in range(NK):
                        nc.tensor.matmul(
                            ps[:64, :],
                            kv_bf[:, k * DK : (k + 1) * DK],
                            xT[:, k * S + sg * SBLK : k * S + (sg + 1) * SBLK],
                            start=(k == 0),
                            stop=(k == NK - 1),
                        )
                    nc.vector.tensor_scalar_add(
                        KT[:64, sg * SBLK : (sg + 1) * SBLK], ps[:64, :], bk_t[:]
                    )
                nc.sync.dma_start(KT[64:128, :], KT[:64, :])

                for t in range(NT):
                    ps = ps_proj.tile([128, SBLK], F32, tag="p")
                    for k in range(NK):
                        nc.tensor.matmul(
                            ps[:, :DK],
                            xT[:, k * S + t * 128 : k * S + (t + 1) * 128],
                            kv_bf[:, 512 + k * DK : 512 + (k + 1) * DK],
                            start=(k == 0),
                            stop=(k == NK - 1),
                        )
                    nc.vector.tensor_copy(Vb[:, t * 65 : t * 65 + DK], ps[:, :DK])
                nc.vector.memset(Vb[:].rearrange("p (t c) -> p t c", c=65)[:, :, DK], 1.0)

                for m in range(2):
                    for sg in range(NSB):
                        ps = ps_proj.tile([128, SBLK], F32, tag="p")
                        for k in range(NK):
                            nc.tensor.matmul(
                                ps[:],
                                wq_bf[:, k * GQ + m * 128 : k * GQ + (m + 1) * 128],
                                xT[:, k * S + sg * SBLK : k * S + (sg + 1) * SBLK],
                                start=(k == 0),
                                stop=(k == NK - 1),
                            )
                        nc.vector.tensor_scalar_add(
                            QT[:, m * S + sg * SBLK : m * S + (sg + 1) * SBLK], ps[:], bq_t[:, m : m + 1]
                        )

                # ---- attention + output, per query super-block ----
                PT = acts.tile([128, NT * H * SBLK], BF)  # col = (t*H + h)*SBLK + s_local
                ctx_sb = acts.tile([128, 4 * GQ], BF)  # col = sc*GQ + h*DK + d
                ctxT_sb = acts.tile([128, 2 * SBLK], BF)  # col = cj*SBLK + sc*128 + s

                for sb in range(NSB):
                    # scores^T + exp, per (token chunk, head pair); the two heads
                    # of a pair sit at SBUF partitions 0-63 / 64-127 and map to
                    # PE row-tiles (0,0) / (64,0), so their matmuls can overlap.
                    for t in range(NT):
                        for p in range(2):
                            sc = ps_sc.tile([128, 2 * SBLK], F32, tag="sc")
                            for hl in range(2):
                                h = 2 * p + hl
                                nc.tensor.matmul(
                                    sc[:, hl * SBLK : (hl + 1) * SBLK],
                                    KT[hl * 64 : (hl + 1) * 64, t * 128 : (t + 1) * 128],
                                    QT[hl * 64 : (hl + 1) * 64,
                                       p * S + sb * SBLK : p * S + (sb + 1) * SBLK],
                                )
                            nc.scalar.activation(
                                PT[:, (t * H + 2 * p) * SBLK : (t * H + 2 * p + 2) * SBLK],
                                sc[:],
                                mybir.ActivationFunctionType.Exp,
                                scale=0.125,
                            )

                    # ctx natural, one head at a time; col 64 of each group = denom
                    for h in range(H):
                        cps = ps_ctx.tile([128, 4 * (DK + 1)], F32, tag="c")
                        for sc_i in range(4):
                            for t in range(NT):
                                nc.tensor.matmul(
                                    cps[:, sc_i * 65 : sc_i * 65 + 65],
                                    PT[:, (t * H + h) * SBLK + sc_i * 128 : (t * H + h) * SBLK + (sc_i + 1) * 128],
                                    Vb[:, t * 65 : (t + 1) * 65],
                                    start=(t == 0),
                                    stop=(t == NT - 1),
                                )
                        rc = xin.tile([128, 4], F32, tag="rc")
                        nc.vector.reciprocal(
                            rc[:], cps[:].rearrange("p (sc c) -> p sc c", c=65)[:, :, DK]
                        )
                        for sc_i in range(4):
                            nc.vector.tensor_scalar_mul(
                                ctx_sb[:, sc_i * GQ + h * DK : sc_i * GQ + (h + 1) * DK],
                                cps[:, sc_i * 65 : sc_i * 65 + DK],
                                rc[:, sc_i : sc_i + 1],
                            )

                    # transpose ctx -> ctxT
                    for cj in range(2):
                        ps = ps_proj.tile([128, 2 * SBLK], BF, tag="p", name="pst")
                        for sc_i in range(4):
                            nc.tensor.transpose(
                                ps[:, sc_i * 128 : (sc_i + 1) * 128],
                                ctx_sb[:, sc_i * GQ + cj * 128 : sc_i * GQ + (cj + 1) * 128],
                                ident_b[:],
                            )
                        nc.vector.tensor_copy(ctxT_sb[:, cj * SBLK : (cj + 1) * SBLK], ps[:, :SBLK])

                    # output projection (partial over this group's 256 dims)
                    for half in range(2):
                        ot = outp.tile([128, 2 * DM], F32, tag="ot")
                        for ci in range(2):
                            sc_i = half * 2 + ci
                            for nb in range(2):
                                ps = ps_proj.tile([128, SBLK], F32, tag="p")
                                for cj in range(2):
                                    nc.tensor.matmul(
                                        ps[:],
                                        ctxT_sb[:, cj * SBLK + sc_i * 128 : cj * SBLK + (sc_i + 1) * 128],
                                        wo_bf[:, cj * DM + nb * SBLK : cj * DM + (nb + 1) * SBLK],
                                        start=(cj == 0),
                                        stop=(cj == 1),
                                    )
                                nc.vector.tensor_copy(
                                    ot[:, ci * DM + nb * SBLK : ci * DM + (nb + 1) * SBLK], ps[:]
                                )
                        row = sb * SBLK + half * 256
                        nc.sync.dma_start(
                            out[row : row + 256, :].rearrange("(c p) d -> p c d", p=128),
                            ot[:].rearrange("p (c d) -> p c d", d=DM),
                        )

            if iters == 1:
                _pipeline()
            else:
                with tc.For_i(0, iters):
                    _pipeline()

    _split_sync_waits(nc)
    return nc


def kernel(x, W_Q, b_Q, W_K, b_K, W_V, b_V, W_O, b_O):
    from concourse.bass_utils import run_bass_kernel_spmd

    x = np.asarray(x, np.float32)
    W_Q, b_Q = np.asarray(W_Q, np.float32), np.asarray(b_Q, np.float32)
    W_K, b_K = np.asarray(W_K, np.float32), np.asarray(b_K, np.float32)
    W_V, b_V = np.asarray(W_V, np.float32), np.asarray(b_V, np.float32)
    W_O, b_O = np.asarray(W_O, np.float32), np.asarray(b_O, np.float32)

    if "nc" not in _CACHED:
        _CACHED["nc"] = _build_nc()
    nc = _CACHED["nc"]

    in_maps = []
    for c in range(8):
        b, g = divmod(c, 4)
        in_maps.append(
            {
                "x": np.ascontiguousarray(x[b]),
                "wq": np.ascontiguousarray(W_Q[:, g * GQ : (g + 1) * GQ]),
                "wk": np.ascontiguousarray(W_K[g]),
                "wv": np.ascontiguousarray(W_V[g]),
                "wo": np.ascontiguousarray(W_O[g * GQ : (g + 1) * GQ, :]),
                "bq": np.ascontiguousarray(b_Q[g * GQ : (g + 1) * GQ]),
                "bk": np.ascontiguousarray(b_K[g]),
            }
        )
    res = run_bass_kernel_spmd(nc, in_maps, list(range(8)))

    out = np.zeros((B, S, DM), np.float32)
    for c in range(8):
        b, g = divmod(c, 4)
        out[b] += res.results[c]["out"]
    # host-side bias terms: b_O, plus b_V's exact contribution
    # (softmax rows sum to 1 -> ctx bias = tile(b_V[g]) per head)
    bv_full = np.concatenate([np.tile(b_V[g], H) for g in range(G)])  # [1024]
    out += (b_O + bv_full @ W_O)[None, None, :]
    return out

